# revision 7
# baseline (speedup 1.0000x reference)
"""Trainium2 kernel for Conv2d_cd (central-difference conv, 3x3, theta=0.7).

Reference math:
    s = sum of 9 shifted views of reflect-padded x  (= 3x3 box filter, reflect pad)
    out = conv3x3_zeropad(s, W) - theta * conv1x1(s, W.sum((2,3)))
        = conv3x3_zeropad(s, W')     with W'[:,:,1,1] -= theta * W.sum((2,3))

Strategy (per NeuronCore, 8 cores data-parallel over batch 16 -> 2 images/core):
  - images stacked on SBUF partition halves: partitions 0:64 = img0 ch, 64:128 = img1 ch
  - H strips of 32 output rows; per strip:
      * DVE separable box filter: horizontal pass fp32 (1x), vertical pass bf16 (2x)
        (folded strips skip the horizontal pass; it is folded into 15-tap weights)
      * conv taps as K=64/M=64 matmuls packed 4-concurrent in PE quadrants via
        tile_position (auto-derived from base partitions), accumulating in PSUM
      * ScalarE evacuates PSUM -> SBUF; strided DMAs store to DRAM
"""

import numpy as np
import ml_dtypes

import concourse.bass as bass
import concourse.bacc as bacc
import concourse.mybir as mybir
from concourse.tile import TileContext
from concourse.bass_utils import run_bass_kernel_spmd

THETA = 0.7
N_CORES = 8
B, C, H, W = 16, 64, 128, 128
BPC = B // N_CORES          # images per core = 2
R = 32                      # strip height (output rows)
NSTRIP = H // R             # 4
WP = W + 4                  # padded width of s/sv buffers (132)
F32 = mybir.dt.float32
BF16 = mybir.dt.bfloat16

# which strips use the folded (15-tap, no horizontal DVE pass) path
FOLDED = [False, False, False, False]


def _host_weights(Wnp: np.ndarray):
    """Compute W' and the tap weight matrices on host (tiny)."""
    Wp = Wnp.astype(np.float64).copy()
    Wp[:, :, 1, 1] -= THETA * Wnp.astype(np.float64).sum(axis=(2, 3))
    # w9[ci, t= ky*3+kx, co] = W'[co, ci, ky, kx]  (lhsT layout, tap-major free dim)
    w9 = np.ascontiguousarray(Wp.transpose(1, 2, 3, 0).reshape(C, 9, C))
    # folded horizontal box: W''[ky, tx] = sum_{kx: max(0,tx-2)<=kx<=min(2,tx)} W'[ky,kx]
    w15 = np.zeros((C, 3, 5, C), np.float64)
    for ky in range(3):
        for tx in range(5):
            for kx in range(max(0, tx - 2), min(2, tx) + 1):
                w15[:, ky, tx, :] += Wp[:, :, ky, kx].T  # [ci, co]
    w9 = w9.reshape(C, 9 * C)
    w15 = w15.reshape(C, 15 * C)
    return (w9.astype(ml_dtypes.bfloat16), w15.astype(ml_dtypes.bfloat16))


def build():
    nc = bacc.Bacc("TRN2", target_bir_lowering=False, debug=False,
                   num_devices=N_CORES)
    x_d = nc.declare_dram_parameter("x", [BPC, C, H, W], F32, isOutput=False)
    w9_d = nc.declare_dram_parameter("w9", [C, 9 * C], BF16, isOutput=False)
    w15_d = nc.declare_dram_parameter("w15", [C, 15 * C], BF16, isOutput=False)
    out_d = nc.declare_dram_parameter("out", [BPC, C, H, W], F32, isOutput=True)

    # partition-major views: (img, ch) -> 128 partitions
    x_pc = x_d.rearrange("i c h w -> (i c) h w")
    # out view for strided stores: [img, ch, g2(16), two(2), (four*w)(512)]
    out_v = out_d.rearrange("i c (g2 two four) w -> i c g2 two (four w)",
                            two=2, four=4)

    with TileContext(nc) as tc:
        with (
            tc.tile_pool(name="wpool", bufs=1) as wpool,
            tc.tile_pool(name="xpool", bufs=2) as xpool,
            tc.tile_pool(name="hpool", bufs=2) as hpool,
            tc.tile_pool(name="spool", bufs=2) as spool,
            tc.tile_pool(name="opool", bufs=2) as opool,
            tc.tile_pool(name="psum", bufs=8, space="PSUM") as ppool,
        ):
            # --- weights: both partition halves get identical copies ---
            w9_sb = wpool.tile([128, 9 * C], BF16)
            w15_sb = wpool.tile([128, 15 * C], BF16)
            for base in (0, 64):
                nc.sync.dma_start(out=w9_sb[base:base + 64, :], in_=w9_d[:])
                nc.sync.dma_start(out=w15_sb[base:base + 64, :], in_=w15_d[:])

            for si in range(NSTRIP):
                r0 = si * R
                folded = FOLDED[si]
                xdtype = BF16 if folded else F32
                xdma = nc.gpsimd if folded else nc.sync

                # ---- load x rows [r0-2, r0+R+2) into 36 slots (slot m = abs row r0-2+m)
                xt = xpool.tile([128, 36 * W], xdtype, tag="xt")
                x3 = xt.rearrange("p (r w) -> p r w", w=W)
                row_lo, row_hi = max(0, r0 - 2), min(H, r0 + R + 2)
                slot_lo = row_lo - (r0 - 2)
                xdma.dma_start(out=x3[:, slot_lo:slot_lo + (row_hi - row_lo), :],
                               in_=x_pc[:, row_lo:row_hi, :])
                if si == 0:
                    # slot 1 = abs row -1 -> reflect = x row 1 ; slot 0 unused
                    xdma.dma_start(out=x3[:, 1:2, :], in_=x_pc[:, 1:2, :])
                    nc.any.memset(x3[:, 0:1, :], 0.0)
                if si == NSTRIP - 1:
                    # slot 34 = abs row 128 -> reflect = x row 126 ; slot 35 unused
                    xdma.dma_start(out=x3[:, 34:35, :], in_=x_pc[:, 126:127, :])
                    nc.any.memset(x3[:, 35:36, :], 0.0)

                if not folded:
                    # ---- horizontal box (fp32): sh = x(w-1)+x(w)+x(w+1), reflect at edges
                    tt = hpool.tile([128, 36 * W], F32, tag="tt")
                    t3 = tt.rearrange("p (r w) -> p r w", w=W)
                    nc.vector.tensor_add(out=t3[:, :, 1:127],
                                         in0=x3[:, :, 0:126], in1=x3[:, :, 2:128])
                    nc.vector.tensor_scalar_mul(out=t3[:, :, 0:1],
                                                in0=x3[:, :, 1:2], scalar1=2.0)
                    nc.vector.tensor_scalar_mul(out=t3[:, :, 127:128],
                                                in0=x3[:, :, 126:127], scalar1=2.0)
                    sh = hpool.tile([128, 36 * W], BF16, tag="sh")
                    nc.vector.tensor_add(out=sh[:], in0=tt[:], in1=xt[:])
                    vsrc = sh
                else:
                    vsrc = xt

                # ---- vertical box (bf16, 2x): s[j] = v[j] + v[j+1] + v[j+2]
                ut = hpool.tile([128, 34 * W], BF16, tag="ut")
                nc.vector.tensor_add(out=ut[:], in0=vsrc[:, 0:34 * W],
                                     in1=vsrc[:, 2 * W:36 * W])
                st = spool.tile([128, 34 * WP], BF16, tag="st")
                s3 = st.rearrange("p (r c) -> p r c", c=WP)
                u3 = ut.rearrange("p (r w) -> p r w", w=W)
                v3 = vsrc.rearrange("p (r w) -> p r w", w=W)
                nc.vector.tensor_add(out=s3[:, :, 2:130],
                                     in0=u3[:, :, :], in1=v3[:, 1:35, :])

                # ---- side columns
                if not folded:
                    # zero-pad columns for the 3-wide taps (cols 1..130 read)
                    nc.any.memset(s3[:, :, 0:2], 0.0)
                    nc.any.memset(s3[:, :, 130:132], 0.0)
                else:
                    # sv buffer: col c = sv[x-col c-2]; taps read cols 0..131.
                    # col1 := col3 (reflect -1 -> +1), col130 := col128 (reflect 128 -> 126)
                    nc.vector.tensor_copy(out=s3[:, :, 1:2], in_=s3[:, :, 3:4])
                    nc.vector.tensor_copy(out=s3[:, :, 130:131], in_=s3[:, :, 128:129])
                    # col0 := -(col3 + col2)   [makes folded s(-1) == 0]
                    nc.vector.tensor_add(out=s3[:, :, 0:1],
                                         in0=s3[:, :, 3:4], in1=s3[:, :, 2:3])
                    nc.scalar.mul(s3[:, :, 0:1], s3[:, :, 0:1], -1.0)
                    # col131 := -(col129 + col128)  [makes folded s(128) == 0]
                    nc.vector.tensor_add(out=s3[:, :, 131:132],
                                         in0=s3[:, :, 129:130], in1=s3[:, :, 128:129])
                    nc.scalar.mul(s3[:, :, 131:132], s3[:, :, 131:132], -1.0)

                # ---- zero-pad rows (conv zero padding at image top/bottom)
                if si == 0:
                    nc.any.memset(s3[:, 0:1, :], 0.0)
                if si == NSTRIP - 1:
                    nc.any.memset(s3[:, 33:34, :], 0.0)

                # ---- conv taps: accumulate into 8 psum banks
                # chunk c (0..7) = out local rows [4c, 4c+4); pair p = c//2
                # PA[p]: img0@0:64 (c even), img1@64:128 ; PB[p]: img0@64:128 (c odd), img1@0:64
                pa = [ppool.tile([128, 512], F32, tag="ps", name=f"pa{si}_{j}")
                      for j in range(4)]
                pb = [ppool.tile([128, 512], F32, tag="ps", name=f"pb{si}_{j}")
                      for j in range(4)]
                ntap = 15 if folded else 9
                wsb = w15_sb if folded else w9_sb
                nkx = 5 if folded else 3
                cofs = 0 if folded else 1
                for t in range(ntap):
                    ky, kx = t // nkx, t % nkx
                    for p in range(4):
                        for (i, c) in ((0, 2 * p), (1, 2 * p), (0, 2 * p + 1),
                                       (1, 2 * p + 1)):
                            ptile = pa[p] if c % 2 == 0 else pb[p]
                            pbase = 64 * i if c % 2 == 0 else 64 * (1 - i)
                            # s local row j = (out local row) + ky ; out local = 4c..4c+4
                            rhs = s3[64 * i:64 * i + 64,
                                     4 * c + ky:4 * c + ky + 4,
                                     kx + cofs:kx + cofs + 128]
                            nc.tensor.matmul(
                                ptile[pbase:pbase + 64, :],
                                wsb[64 * i:64 * i + 64, t * C:(t + 1) * C],
                                rhs,
                                start=(t == 0), stop=(t == ntap - 1),
                                skip_group_check=True,
                            )

                # ---- evacuate psum -> sbuf (ScalarE)
                ot = opool.tile([128, R * W], F32, tag="ot")
                o3 = ot.rearrange("p (c n) -> p c n", n=512)
                for c in range(8):
                    ptile = pa[c // 2] if c % 2 == 0 else pb[c // 2]
                    nc.scalar.copy(out=o3[:, c:c + 1, :], in_=ptile[:])

                # ---- store: 4 strided DMAs (even/odd chunks x partition halves)
                o4 = ot.rearrange("p (c2 two n) -> p c2 two n", two=2, n=512)
                g = 4 * si
                nc.sync.dma_start(out=out_v[0, :, g:g + 4, 0, :],
                                  in_=o4[0:64, :, 0, :])
                nc.sync.dma_start(out=out_v[0, :, g:g + 4, 1, :],
                                  in_=o4[64:128, :, 1, :])
                nc.sync.dma_start(out=out_v[1, :, g:g + 4, 0, :],
                                  in_=o4[64:128, :, 0, :])
                nc.sync.dma_start(out=out_v[1, :, g:g + 4, 1, :],
                                  in_=o4[0:64, :, 1, :])

    nc.compile()
    return nc


_CACHE = {}


def _get_nc():
    if "nc" not in _CACHE:
        _CACHE["nc"] = build()
    return _CACHE["nc"]


def kernel(x: np.ndarray, W: np.ndarray, trace: bool = False):
    x = np.asarray(x, dtype=np.float32)
    w9, w15 = _host_weights(np.asarray(W, dtype=np.float32))
    nc = _get_nc()
    core_ids = list(range(N_CORES))
    in_maps = [
        {"x": np.ascontiguousarray(x[BPC * i:BPC * (i + 1)]),
         "w9": w9, "w15": w15}
        for i in core_ids
    ]
    res = run_bass_kernel_spmd(nc, in_maps, core_ids, trace=trace)
    out = np.concatenate([res.results[i]["out"] for i in core_ids], axis=0)
    if trace:
        kernel.last_exec_time_ns = res.exec_time_ns
    return out


kernel.last_exec_time_ns = None
